# revision 1
# baseline (speedup 1.0000x reference)
"""Trainium2 Bass kernel for the GRU decoder problem.

Strategy
--------
Data-parallel over 8 NeuronCores: batch 8192 -> 1024 per core; GRU/FC
weights replicated.  Everything on-chip is feature-major (features on
SBUF partitions, batch along the free dim) so that the recurrent
matmuls contract over the partition dim with weights stationary.

Math (per core, B=1024, exact reformulation of the reference):
    h0   = fc2(fc1(latent))
    per step t = 0..118:
        rz   = h @ Whh_rz^T + x @ Wih_rz^T + b          (step 0)
        rz   = h @ G_rz + b_f                           (t>=1, fused: x_t = pred_t = h_t @ C + c_b)
        r,z  = sigmoid(rz)
        n    = tanh(x @ Wih_n^T + b_ih_n + r*(h @ Whh_n^T + b_hh_n))
        h    = n + z*(h - n)
        pred = h @ C + c_b        with C = h1_w^T @ h2_w^T  (exact fusion)
        y[:, t+1] = pred;  x = pred
Precision: fp16 operands, fp32 PSUM accumulate, fp32 biases applied on
the fp32 PSUM values (verified rel_l2 ~ 1e-4 vs fp32 reference).
"""

import sys

import numpy as np

if "/opt/trn_rl_repo" not in sys.path:
    sys.path.insert(0, "/opt/trn_rl_repo")

from contextlib import ExitStack

import concourse.bass as bass
import concourse.tile as tile
from concourse import bacc, mybir
from concourse.bass import ds, ts
from concourse.bass_utils import run_bass_kernel_spmd
from concourse.masks import make_identity

P = 128
H = 512
HK = 4            # H // 128 contraction chunks
A = 40            # alphabet
AT = 48           # transpose-padded alphabet rows
B = 1024          # batch per core
NCOL = 512        # batch chunk (matmul N / PSUM bank)
NB = 2            # chunks per core
T = 119           # recurrent steps
SG = 17           # steps per output DMA group
NG = 7            # groups (7*17 = 119)
N_CORES = 8

F16 = mybir.dt.float16
F32 = mybir.dt.float32

AF = mybir.ActivationFunctionType
OP = mybir.AluOpType


def _emit(nc, bench=False, repeat=1):
    """Emit the full Tile program. Returns nothing; tensors are declared here."""
    # ---- DRAM I/O ------------------------------------------------------
    d_latT = nc.dram_tensor("latT", [P, B], F16, kind="ExternalInput").ap()
    d_fc1T = nc.dram_tensor("fc1T", [P, H], F16, kind="ExternalInput").ap()
    d_fc2T = nc.dram_tensor("fc2T", [P, HK, H], F16, kind="ExternalInput").ap()
    d_grz = nc.dram_tensor("grz", [P, HK, 2 * H], F16, kind="ExternalInput").ap()
    d_whhrz = nc.dram_tensor("whhrz", [P, HK, 2 * H], F16, kind="ExternalInput").ap()
    d_whhn = nc.dram_tensor("whhn", [P, HK, H], F16, kind="ExternalInput").ap()
    d_wihrz = nc.dram_tensor("wihrz", [P, 2 * H], F16, kind="ExternalInput").ap()
    d_wihn = nc.dram_tensor("wihn", [P, H], F16, kind="ExternalInput").ap()
    d_cmat = nc.dram_tensor("cmat", [P, HK, A], F16, kind="ExternalInput").ap()
    d_brz0 = nc.dram_tensor("brz0", [P, 8], F32, kind="ExternalInput").ap()
    d_brzf = nc.dram_tensor("brzf", [P, 8], F32, kind="ExternalInput").ap()
    d_bhhn = nc.dram_tensor("bhhn", [P, HK], F32, kind="ExternalInput").ap()
    d_bihn = nc.dram_tensor("bihn", [P, HK], F32, kind="ExternalInput").ap()
    d_fc1b = nc.dram_tensor("fc1b", [P, HK], F32, kind="ExternalInput").ap()
    d_fc2b = nc.dram_tensor("fc2b", [P, HK], F32, kind="ExternalInput").ap()
    d_cb = nc.dram_tensor("cb", [A, 1], F32, kind="ExternalInput").ap()
    if bench:
        d_y = nc.dram_tensor("ybench", [B, T, A], F32, kind="Internal").ap()
        d_dummy = nc.dram_tensor("y", [A, 1], F32, kind="ExternalOutput").ap()
    else:
        d_y = nc.dram_tensor("y", [B, T, A], F32, kind="ExternalOutput").ap()

    with tile.TileContext(nc) as tc, ExitStack() as ctx:
        const = ctx.enter_context(tc.tile_pool(name="const", bufs=1))
        state = ctx.enter_context(tc.tile_pool(name="state", bufs=1))
        ew = ctx.enter_context(tc.tile_pool(name="ew", bufs=4))
        rzp = ctx.enter_context(tc.tile_pool(name="rzp", bufs=6))
        stg = ctx.enter_context(tc.tile_pool(name="stg", bufs=2))
        # PSUM: gates(rz+gxn shared) 4 + ghn 2 + pred 2 = 8 banks
        ps_g = ctx.enter_context(tc.tile_pool(name="ps_g", bufs=4, space="PSUM"))
        ps_h = ctx.enter_context(tc.tile_pool(name="ps_h", bufs=2, space="PSUM"))
        ps_p = ctx.enter_context(tc.tile_pool(name="ps_p", bufs=2, space="PSUM"))

        # ---- load constants/weights ------------------------------------
        def cload(name, dram, shape, dtype):
            t = const.tile(shape, dtype, tag=name, name=name)
            nc.sync.dma_start(t[:], dram)
            return t

        latT = cload("latT", d_latT, [P, B], F16)
        fc1T = cload("fc1T", d_fc1T, [P, H], F16)
        fc2T = cload("fc2T", d_fc2T, [P, HK, H], F16)
        grz = cload("grz", d_grz, [P, HK, 2 * H], F16)
        whhrz = cload("whhrz", d_whhrz, [P, HK, 2 * H], F16)
        whhn = cload("whhn", d_whhn, [P, HK, H], F16)
        wihrz = cload("wihrz", d_wihrz, [P, 2 * H], F16)
        wihn = cload("wihn", d_wihn, [P, H], F16)
        cmat = cload("cmat", d_cmat, [P, HK, A], F16)
        brz0 = cload("brz0", d_brz0, [P, 8], F32)
        brzf = cload("brzf", d_brzf, [P, 8], F32)
        bhhn = cload("bhhn", d_bhhn, [P, HK], F32)
        bihn = cload("bihn", d_bihn, [P, HK], F32)
        fc1b = cload("fc1b", d_fc1b, [P, HK], F32)
        fc2b = cload("fc2b", d_fc2b, [P, HK], F32)
        cb = cload("cb", d_cb, [A, 1], F32)

        # ---- persistent state ------------------------------------------
        # h is double-buffered by step parity: step t reads hbuf[t%2],
        # writes hbuf[(t+1)%2] (the ghn matmuls of tile m4 read ALL k-chunks,
        # so in-place updates would corrupt them).
        hbuf = [
            [state.tile([P, HK, NCOL], F16, tag=f"h{p}{c}", name=f"h{p}{c}")
             for c in range(NB)]
            for p in range(2)
        ]
        x16 = [state.tile([P, NCOL], F16, tag=f"x{c}", name=f"x{c}") for c in range(NB)]
        for c in range(NB):
            nc.gpsimd.memset(x16[c][:], 0.0)
            nc.gpsimd.memset(x16[c][:A, :], -16.0)
            nc.gpsimd.memset(x16[c][0:1, :], 16.0)

        # ---- h0 = fc2(fc1(latent)) -------------------------------------
        h16 = hbuf[0]
        for c in range(NB):
            t0 = state.tile([P, HK, NCOL], F16, tag=f"t0_{c}", name=f"t0_{c}")
            for m in range(HK):
                pt = ps_g.tile([P, NCOL], F32, tag="g", name="ps_g")
                nc.tensor.matmul(
                    pt[:], fc1T[:, ts(m, P)], latT[:, ts(c, NCOL)],
                    start=True, stop=True,
                )
                nc.scalar.activation(
                    t0[:, m, :], pt[:], AF.Identity, bias=fc1b[:, m : m + 1]
                )
            for m in range(HK):
                ph = ps_h.tile([P, NCOL], F32, tag="h", name="ps_h")
                for k in range(HK):
                    nc.tensor.matmul(
                        ph[:], fc2T[:, k, ts(m, P)], t0[:, k, :],
                        start=(k == 0), stop=(k == HK - 1),
                    )
                nc.scalar.activation(
                    h16[c][:, m, :], ph[:], AF.Identity, bias=fc2b[:, m : m + 1]
                )

        # elementwise tiles are per (step, chunk, m4); tags share pool slots
        r_t = {}
        z_t = {}
        stage = [None]

        def emit_gates(t, c):
            hr = hbuf[t % 2][c]        # state entering step t
            hw = hbuf[(t + 1) % 2][c]  # state leaving step t
            # r/z: 8 feature tiles of 128
            for m in range(8):
                pt = ps_g.tile([P, NCOL], F32, tag="g", name="ps_g")
                w = grz if t > 0 else whhrz
                for k in range(HK):
                    nc.tensor.matmul(
                        pt[:], w[:, k, ts(m, P)], hr[:, k, :],
                        start=(k == 0), stop=(k == HK - 1 and t > 0),
                    )
                if t == 0:
                    nc.tensor.matmul(
                        pt[:], wihrz[:, ts(m, P)], x16[c][:],
                        start=False, stop=True,
                    )
                bias = (brzf if t > 0 else brz0)[:, m : m + 1]
                dst = rzp.tile([P, NCOL], F16, tag=("r" if m < 4 else "z"), name="rz")
                nc.scalar.activation(dst[:], pt[:], AF.Sigmoid, bias=bias)
                if m < 4:
                    r_t[(c, m)] = dst
                else:
                    z_t[(c, m - 4)] = dst

            # n gate per 128-feature tile
            for m in range(HK):
                pg = ps_h.tile([P, NCOL], F32, tag="h", name="ps_h")
                for k in range(HK):
                    nc.tensor.matmul(
                        pg[:], whhn[:, k, ts(m, P)], hr[:, k, :],
                        start=(k == 0), stop=(k == HK - 1),
                    )
                px = ps_g.tile([P, NCOL], F32, tag="g", name="ps_g")
                nc.tensor.matmul(px[:], wihn[:, ts(m, P)], x16[c][:],
                                 start=True, stop=True)
                u = ew.tile([P, NCOL], F16, tag="u", name="u")
                nc.vector.scalar_tensor_tensor(
                    u[:], pg[:], bhhn[:, m : m + 1], r_t[(c, m)][:],
                    OP.add, OP.mult,
                )
                v = ew.tile([P, NCOL], F16, tag="v", name="v")
                nc.vector.tensor_tensor(v[:], u[:], px[:], OP.add)
                nt = ew.tile([P, NCOL], F16, tag="n", name="n")
                nc.scalar.activation(nt[:], v[:], AF.Tanh, bias=bihn[:, m : m + 1])
                d = ew.tile([P, NCOL], F16, tag="d", name="d")
                nc.vector.tensor_tensor(d[:], hr[:, m, :], nt[:], OP.subtract)
                e = ew.tile([P, NCOL], F16, tag="e", name="e")
                nc.vector.tensor_tensor(e[:], z_t[(c, m)][:], d[:], OP.mult)
                nc.vector.tensor_tensor(hw[:, m, :], nt[:], e[:], OP.add)

        def emit_pred(t, c):
            s = t % SG
            g = t // SG
            if c == 0 and s == 0:
                stage[0] = stg.tile([P, 8, SG, AT], F16, tag="st", name="stage")
            hn = hbuf[(t + 1) % 2][c]  # state AFTER step t
            pp = ps_p.tile([A, NCOL], F32, tag="p", name="ps_p")
            for k in range(HK):
                nc.tensor.matmul(
                    pp[:], cmat[:, k, :], hn[:, k, :],
                    start=(k == 0), stop=(k == HK - 1),
                )
            # pred -> next x (fp16), rows A..P stay zero
            nc.scalar.activation(x16[c][:A, :], pp[:], AF.Identity, bias=cb[:])
            # batch-major transpose via DMA xbar straight into the stage
            for j in range(4):
                bt = c * 4 + j
                nc.sync.dma_start_transpose(
                    stage[0][:, bt, s, :], x16[c][:AT, ts(j, P)]
                )
            if c == NB - 1 and s == SG - 1:
                for bt in range(8):
                    nc.gpsimd.dma_start(
                        d_y[ts(bt, P), ts(g, SG), :],
                        stage[0][:, bt, :, :A],
                    )

        if bench:
            nc.sync.dma_start(d_dummy[:], cb[:])

        # software pipeline: gates(t+1) slotted between pred(t, c0) and pred(t, c1)
        for rep in range(repeat):
            emit_gates(0, 0)
            emit_gates(0, 1)
            for t in range(T):
                emit_pred(t, 0)
                if t + 1 < T:
                    emit_gates(t + 1, 0)
                emit_pred(t, 1)
                if t + 1 < T:
                    emit_gates(t + 1, 1)


_CACHE = {}


def _build(bench=False, repeat=1):
    key = f"nc_bench{repeat}" if bench else "nc"
    if key in _CACHE:
        return _CACHE[key]
    nc = bacc.Bacc(
        "TRN2",
        target_bir_lowering=False,
        debug=False,
        enable_asserts=False,
        num_devices=1 if bench else N_CORES,
    )
    _emit(nc, bench=bench, repeat=repeat)
    nc.compile()
    _CACHE[key] = nc
    return nc


def _prep_inputs(latent, fc1_w, fc1_b, fc2_w, fc2_b, W_ih, W_hh, b_ih, b_hh,
                 h1_w, h1_b, h2_w, h2_b):
    """Host-side weight fusion / layout prep. Returns per-core input maps."""
    f64 = np.float64

    def kchunk(wT, cols):
        # [H, cols] -> [P, HK, cols]
        return np.ascontiguousarray(
            wT.reshape(HK, P, cols).transpose(1, 0, 2)
        ).astype(np.float16)

    C = h1_w.T.astype(f64) @ h2_w.T.astype(f64)            # [H, A]
    c_b = h1_b.astype(f64) @ h2_w.T.astype(f64) + h2_b.astype(f64)  # [A]
    WihT = W_ih.T.astype(f64)                               # [A, 3H]
    G_rz = W_hh.T[:, : 2 * H].astype(f64) + C @ WihT[:, : 2 * H]
    b_rz0 = (b_ih[: 2 * H] + b_hh[: 2 * H]).astype(f64)
    b_rzf = b_rz0 + c_b @ WihT[:, : 2 * H]

    def pad_rows(w, rows):
        out = np.zeros((rows, w.shape[1]), np.float16)
        out[: w.shape[0]] = w.astype(np.float16)
        return out

    common = {
        "fc1T": np.ascontiguousarray(fc1_w.T).astype(np.float16),
        "fc2T": kchunk(fc2_w.T.astype(np.float32), H),
        "grz": kchunk(G_rz.astype(np.float32), 2 * H),
        "whhrz": kchunk(W_hh.T[:, : 2 * H].astype(np.float32), 2 * H),
        "whhn": kchunk(W_hh.T[:, 2 * H :].astype(np.float32), H),
        "wihrz": pad_rows(WihT[:, : 2 * H].astype(np.float32), P),
        "wihn": pad_rows(WihT[:, 2 * H :].astype(np.float32), P),
        "cmat": kchunk(C.astype(np.float32), A),
        "brz0": np.ascontiguousarray(
            b_rz0.astype(np.float32).reshape(8, P).T),
        "brzf": np.ascontiguousarray(
            b_rzf.astype(np.float32).reshape(8, P).T),
        "bhhn": np.ascontiguousarray(
            b_hh[2 * H :].astype(np.float32).reshape(HK, P).T),
        "bihn": np.ascontiguousarray(
            b_ih[2 * H :].astype(np.float32).reshape(HK, P).T),
        "fc1b": np.ascontiguousarray(fc1_b.astype(np.float32).reshape(HK, P).T),
        "fc2b": np.ascontiguousarray(fc2_b.astype(np.float32).reshape(HK, P).T),
        "cb": c_b.astype(np.float32).reshape(A, 1),
    }
    in_maps = []
    for c in range(N_CORES):
        m = dict(common)
        m["latT"] = np.ascontiguousarray(
            latent[c * B : (c + 1) * B].T
        ).astype(np.float16)
        in_maps.append(m)
    return in_maps


def run(inputs, **kwargs):
    """Build (cached), run on 8 cores, return (y_full, BassKernelResults)."""
    nc = _build()
    in_maps = _prep_inputs(**inputs)
    res = run_bass_kernel_spmd(nc, in_maps, core_ids=list(range(N_CORES)), **kwargs)
    BF = inputs["latent"].shape[0]
    y = np.empty((BF, T + 1, A), np.float32)
    y[:, 0, :] = -16.0
    y[:, 0, 0] = 16.0
    for c in range(N_CORES):
        y[c * B : (c + 1) * B, 1:, :] = res.results[c]["y"]
    return y, res


def kernel(**inputs):
    inputs = {k: np.asarray(v) for k, v in inputs.items()}
    y, _ = run(inputs)
    return y

